# revision 8
# baseline (speedup 1.0000x reference)
"""Conv2d(128->256, 3x3, pad 1, stride 1) on 32x56x56 fp32, for 8 trn2 cores.

Strategy: data-parallel over batch N=32 -> 4 images/core. Per core an
implicit-GEMM conv: C_in=128 is the partition (contraction) dim; for each
(kh, kw) tap a [128ci x 128co] weight tile multiplies a shifted window of the
zero-padded input image held in SBUF, accumulating into PSUM over the 9 taps.
Output rows are processed in chunks of 8 (free dim 8*56=448 <= 512 PSUM bank).
Matmuls run in float16 (~2.6e-4 rel err) with fp32 PSUM accumulate. The 504-
matmul stream runs at the 448-cycle/2.4GHz issue floor, so the win is all in
the head and tail:

Head: weight half-0 is split into its own tiles (taps 0-2, taps 3-8) so the
first LDWEIGHTS gates on a 98KB transfer, not the full weight stream. Image 0
is loaded in 4 pieces split across the sync and gpsimd rings, with chunk 0's
padded rows in a dedicated small tile so its 9 matmuls can start as soon as
x rows 0-8 land (~7.6us) instead of waiting for the whole top half. A short
3-matmul warmup bridges PE busy-ness from ~6.9us for the HAM clock gate.

Tail: the final chunk is drained in four 2-row pieces, two on DVE + two on
ACT (scalar.add), with output DMAs alternating sync/gpsimd rings, so the
last-chunk drain+DMA serialization is ~1.4us instead of ~3us.

Rings: SP(sync) carries x + half-1 fine-grained outputs, ACT(scalar) carries
weights/bias + half-0 outputs, SWDGE(gpsimd) carries image-0 x pieces and
half-1 bulk outputs.
"""
import numpy as np
from contextlib import ExitStack

N_FULL, C_IN, H, W = 32, 128, 56, 56
C_OUT, KS = 256, 3
N_CORES = 8
N_PER = N_FULL // N_CORES          # 4 images per core
HP = H + 2                          # 58 padded
PIX = H * W                         # 3136
ROWS = 8                            # output rows per psum chunk
RC = H // ROWS                      # 7 chunks
NF = ROWS * W                       # 448 free elems per matmul

T_ROWS = 34                         # xpad_top: padded rows 0..33  (chunks 0-3)
B_ROWS = 26                         # xpad_bot: padded rows 32..57 (chunks 4-6)
XT_R = 33                           # x rows 0..32 feed top interior
XT_A = 17                           # images 1-3 first sub-DMA: x rows 0..16
XT_B = XT_R - XT_A                  # images 1-3 second sub-DMA: x rows 17..32
XB_R = 25                           # x rows 31..55 feed bottom interior

# image-0 head pieces
P1_R = 9                            # x rows 0..8   (sync)   -> chunk 0
P2_R = 8                            # x rows 9..16  (gpsimd)
P3_R = 16                           # x rows 17..32 (sync)
Q0_ROWS = 10                        # xq0: padded rows 0..9   (chunk 0)
Q1_ROWS = 26                        # xq1: padded rows 8..33  (chunks 1-3)

_CACHE = {}


def _build():
    import concourse.tile as tile
    from concourse import mybir, bacc

    f32 = mybir.dt.float32
    f16 = mybir.dt.float16

    nc = bacc.Bacc("TRN2", target_bir_lowering=False, debug=False)
    x_d = nc.dram_tensor("x", [N_PER, C_IN, H, W], f16, kind="ExternalInput").ap()
    # host-pretransposed: [ci, half, k, co_half] (half-major, contiguous per half)
    w_d = nc.dram_tensor("w", [C_IN, 2, KS * KS, 128], f16, kind="ExternalInput").ap()
    b_d = nc.dram_tensor("b", [C_OUT], f32, kind="ExternalInput").ap()
    y_d = nc.dram_tensor("y", [N_PER, C_OUT, H, W], f32, kind="ExternalOutput").ap()

    with tile.TileContext(nc) as tc:
        with ExitStack() as ctx:
            wp = ctx.enter_context(tc.tile_pool(name="wp", bufs=1))
            xrawta = ctx.enter_context(tc.tile_pool(name="xrawta", bufs=1))
            xrawtb = ctx.enter_context(tc.tile_pool(name="xrawtb", bufs=1))
            xrawb = ctx.enter_context(tc.tile_pool(name="xrawb", bufs=1))
            xr0b = ctx.enter_context(tc.tile_pool(name="xr0b", bufs=1))
            xq0p = ctx.enter_context(tc.tile_pool(name="xq0p", bufs=1))
            xq1p = ctx.enter_context(tc.tile_pool(name="xq1p", bufs=1))
            xq1bp = ctx.enter_context(tc.tile_pool(name="xq1bp", bufs=1))
            xpadt = ctx.enter_context(tc.tile_pool(name="xpadt", bufs=2))
            xpadb = ctx.enter_context(tc.tile_pool(name="xpadb", bufs=2))
            pp = ctx.enter_context(tc.tile_pool(name="pp", bufs=4, space="PSUM"))
            op = ctx.enter_context(tc.tile_pool(name="op", bufs=2))

            # Weight tiles split per 3 taps; weight pieces ride the sync
            # ring interleaved with image-0's x pieces (the sync ring sustains
            # ~190GB/s from ~8us; the scalar ring is slower in the head).
            w0a = wp.tile([C_IN, 3 * 128], f16)      # half 0, taps 0-2
            w0b1 = wp.tile([C_IN, 3 * 128], f16)     # half 0, taps 3-5
            w0b2 = wp.tile([C_IN, 3 * 128], f16)     # half 0, taps 6-8
            w1 = wp.tile([C_IN, KS * KS * 128], f16)  # half 1, all taps

            # sync ring: p1, w0a, w0b1, p2, w0b2, p3 -- each piece lands just
            # ahead of the warm-rate matmul that first needs it.
            xp1 = xrawta.tile([C_IN, P1_R * W], f16)
            nc.sync.dma_start(xp1[:], x_d[0, :, 0:P1_R, :].rearrange("c h w -> c (h w)"))
            nc.sync.dma_start(
                w0a[:], w_d[:, 0, 0:3].rearrange("ci k co -> ci (k co)")
            )
            nc.sync.dma_start(
                w0b1[:], w_d[:, 0, 3:6].rearrange("ci k co -> ci (k co)")
            )
            xp2 = xr0b.tile([C_IN, P2_R * W], f16)
            nc.sync.dma_start(xp2[:], x_d[0, :, P1_R : P1_R + P2_R, :].rearrange("c h w -> c (h w)"))
            nc.sync.dma_start(
                w0b2[:], w_d[:, 0, 6:9].rearrange("ci k co -> ci (k co)")
            )
            xp3 = xrawtb.tile([C_IN, P3_R * W], f16)
            nc.sync.dma_start(xp3[:], x_d[0, :, 17:33, :].rearrange("c h w -> c (h w)"))

            # scalar ring: bias then weight half 1 (needed from ~22us)
            bias_sb = wp.tile([128, 2], f32)
            nc.scalar.dma_start(bias_sb[:], b_d.rearrange("(h p) -> p h", h=2))
            nc.scalar.dma_start(
                w1[:], w_d[:, 1].rearrange("ci k co -> ci (k co)")
            )

            def lhsT_for(half, k):
                if half == 1:
                    return w1[:, k * 128 : (k + 1) * 128]
                t = (w0a, w0b1, w0b2)[k // 3]
                return t[:, (k % 3) * 128 : (k % 3 + 1) * 128]

            # gpsimd queue: warmup memset first (gates the PE warmup), then
            # the bottom x piece on the SWDGE ring (high latency, needed late),
            # then the image-0 pad memsets (no data deps).
            wu = wp.tile([128, 448], f16)
            nc.gpsimd.memset(wu[:], 0.0)
            xp4 = xrawb.tile([C_IN, XB_R * W], f16)
            nc.gpsimd.dma_start(xp4[:], x_d[0, :, 31 : 31 + XB_R, :].rearrange("c h w -> c (h w)"))

            # PE warmup: 9 dummy matmuls run gaplessly from ~6.9us so the HAM
            # SHORT window sees a full busy window and opens the clock gate at
            # ~10.3us, exactly when the real stream takes over warm.
            wups = pp.tile([128, NF], f32, tag="ps")
            for _ in range(9):
                nc.tensor.matmul(wups[:], wu[:, 0:128], wu[:], start=True, stop=True)

            # image-0 padded tiles, one per chunk-group so each chunk gates
            # only on its own pieces: xq0 = padded rows 0..9 (chunk 0),
            # xq1a = rows 8..17 (chunk 1), xq1b = rows 16..33 (chunks 2-3),
            # xpb0 = rows 32..57 (chunks 4-6).
            xq0 = xq0p.tile([C_IN, Q0_ROWS * HP], f16)
            xq03 = xq0[:].rearrange("p (a b) -> p a b", a=Q0_ROWS)
            nc.gpsimd.memset(xq03[:, 0, :], 0.0)
            nc.gpsimd.memset(xq03[:, 1:Q0_ROWS, 0:1], 0.0)
            nc.gpsimd.memset(xq03[:, 1:Q0_ROWS, HP - 1 : HP], 0.0)
            nc.vector.tensor_copy(
                xq03[:, 1:Q0_ROWS, 1 : 1 + W],
                xp1[:].rearrange("p (a b) -> p a b", a=P1_R),
            )

            xq1a = xq1p.tile([C_IN, 10 * HP], f16)
            xq1a3 = xq1a[:].rearrange("p (a b) -> p a b", a=10)
            nc.gpsimd.memset(xq1a3[:, :, 0:1], 0.0)
            nc.gpsimd.memset(xq1a3[:, :, HP - 1 : HP], 0.0)
            # padded rows 8..17 = x rows 7..16: 2 rows from xp1, 8 from xp2
            nc.vector.tensor_copy(
                xq1a3[:, 0:2, 1 : 1 + W],
                xp1[:].rearrange("p (a b) -> p a b", a=P1_R)[:, 7:9, :],
            )
            nc.vector.tensor_copy(
                xq1a3[:, 2:10, 1 : 1 + W],
                xp2[:].rearrange("p (a b) -> p a b", a=P2_R),
            )

            xq1b = xq1bp.tile([C_IN, 18 * HP], f16)
            xq1b3 = xq1b[:].rearrange("p (a b) -> p a b", a=18)
            nc.gpsimd.memset(xq1b3[:, :, 0:1], 0.0)
            nc.gpsimd.memset(xq1b3[:, :, HP - 1 : HP], 0.0)
            # padded rows 16..33 = x rows 15..32: 2 rows from xp2, 16 from xp3
            nc.vector.tensor_copy(
                xq1b3[:, 0:2, 1 : 1 + W],
                xp2[:].rearrange("p (a b) -> p a b", a=P2_R)[:, 6:8, :],
            )
            nc.vector.tensor_copy(
                xq1b3[:, 2:18, 1 : 1 + W],
                xp3[:].rearrange("p (a b) -> p a b", a=P3_R),
            )

            xpb0 = xpadb.tile([C_IN, B_ROWS * HP], f16)
            xpb03 = xpb0[:].rearrange("p (a b) -> p a b", a=B_ROWS)
            nc.gpsimd.memset(xpb03[:, B_ROWS - 1, :], 0.0)
            nc.gpsimd.memset(xpb03[:, 0 : B_ROWS - 1, 0:1], 0.0)
            nc.gpsimd.memset(xpb03[:, 0 : B_ROWS - 1, HP - 1 : HP], 0.0)
            nc.vector.tensor_copy(
                xpb03[:, 0 : B_ROWS - 1, 1 : 1 + W],
                xp4[:].rearrange("p (a b) -> p a b", a=XB_R),
            )

            for n in range(N_PER):
                if n == 0:
                    xpt3 = None
                    xpb3 = xpb03
                else:
                    # images 1-3: top interior in two slices (pipelined with
                    # the previous image's compute via bufs=1 staging pools)
                    xrta = xrawta.tile([C_IN, XT_A * W], f16)
                    nc.sync.dma_start(xrta[:], x_d[n, :, 0:XT_A, :].rearrange("c h w -> c (h w)"))
                    xrtb = xrawtb.tile([C_IN, XT_B * W], f16)
                    nc.sync.dma_start(xrtb[:], x_d[n, :, XT_A:XT_R, :].rearrange("c h w -> c (h w)"))
                    xrb = xrawb.tile([C_IN, XB_R * W], f16)
                    nc.sync.dma_start(xrb[:], x_d[n, :, 31 : 31 + XB_R, :].rearrange("c h w -> c (h w)"))

                    xpt = xpadt.tile([C_IN, T_ROWS * HP], f16)
                    xpt3 = xpt[:].rearrange("p (a b) -> p a b", a=T_ROWS)
                    nc.vector.memset(xpt3[:, 0, :], 0.0)
                    nc.vector.memset(xpt3[:, 1:T_ROWS, 0:1], 0.0)
                    nc.vector.memset(xpt3[:, 1:T_ROWS, HP - 1 : HP], 0.0)
                    nc.vector.tensor_copy(
                        xpt3[:, 1 : 1 + XT_A, 1 : 1 + W],
                        xrta[:].rearrange("p (a b) -> p a b", a=XT_A),
                    )
                    nc.vector.tensor_copy(
                        xpt3[:, 1 + XT_A : 1 + XT_R, 1 : 1 + W],
                        xrtb[:].rearrange("p (a b) -> p a b", a=XT_B),
                    )

                    xpb = xpadb.tile([C_IN, B_ROWS * HP], f16)
                    xpb3 = xpb[:].rearrange("p (a b) -> p a b", a=B_ROWS)
                    nc.vector.memset(xpb3[:, B_ROWS - 1, :], 0.0)
                    nc.vector.memset(xpb3[:, 0 : B_ROWS - 1, 0:1], 0.0)
                    nc.vector.memset(xpb3[:, 0 : B_ROWS - 1, HP - 1 : HP], 0.0)
                    nc.vector.tensor_copy(
                        xpb3[:, 0 : B_ROWS - 1, 1 : 1 + W],
                        xrb[:].rearrange("p (a b) -> p a b", a=XB_R),
                    )

                out_sb = op.tile([128, 2 * PIX], f32)
                last_img = n == N_PER - 1
                for half in range(2):
                    for rc in range(RC):
                        if last_img and half == 1 and rc == RC - 1:
                            # final chunk as two 4-row PSUM groups: sub-0
                            # drains+stores while sub-1 still computes, so the
                            # post-last-matmul tail covers only 4 rows.
                            HNF = NF // 2      # 224
                            QNF = NF // 4      # 112
                            for sub in range(2):
                                pss = pp.tile([128, HNF], f32)
                                for kh in range(KS):
                                    for kw in range(KS):
                                        k = kh * KS + kw
                                        lr = (rc - 4) * ROWS + sub * 4 + kh
                                        nc.tensor.matmul(
                                            pss[:], lhsT_for(half, k),
                                            xpb3[:, lr : lr + 4, kw : kw + W],
                                            start=(k == 0), stop=(k == KS * KS - 1),
                                        )
                                lo = half * PIX + rc * NF + sub * HNF
                                r0 = rc * ROWS + sub * 4
                                if sub == 0:
                                    nc.vector.tensor_scalar_add(
                                        out_sb[:, lo : lo + HNF], pss[:],
                                        bias_sb[:, half : half + 1],
                                    )
                                    nc.sync.dma_start(
                                        y_d[n, 128:256, r0 : r0 + 4, :]
                                        .rearrange("c h w -> c (h w)"),
                                        out_sb[:, lo : lo + HNF],
                                    )
                                else:
                                    nc.vector.tensor_scalar_add(
                                        out_sb[:, lo : lo + QNF],
                                        pss[:, 0:QNF],
                                        bias_sb[:, half : half + 1],
                                    )
                                    nc.sync.dma_start(
                                        y_d[n, 128:256, r0 : r0 + 2, :]
                                        .rearrange("c h w -> c (h w)"),
                                        out_sb[:, lo : lo + QNF],
                                    )
                                    nc.scalar.add(
                                        out_sb[:, lo + QNF : lo + HNF],
                                        pss[:, QNF:HNF],
                                        bias_sb[:, half : half + 1],
                                    )
                                    nc.scalar.dma_start(
                                        y_d[n, 128:256, r0 + 2 : r0 + 4, :]
                                        .rearrange("c h w -> c (h w)"),
                                        out_sb[:, lo + QNF : lo + HNF],
                                    )
                            continue
                        ps = pp.tile([128, NF], f32)
                        for kh in range(KS):
                            for kw in range(KS):
                                k = kh * KS + kw
                                lhsT = lhsT_for(half, k)
                                if n == 0:
                                    if rc == 0:
                                        rhs = xq03[:, kh : kh + ROWS, kw : kw + W]
                                    elif rc == 1:
                                        rhs = xq1a3[:, kh : kh + ROWS, kw : kw + W]
                                    elif rc < 4:
                                        lr = (rc - 2) * ROWS + kh
                                        rhs = xq1b3[:, lr : lr + ROWS, kw : kw + W]
                                    else:
                                        lr = (rc - 4) * ROWS + kh
                                        rhs = xpb3[:, lr : lr + ROWS, kw : kw + W]
                                else:
                                    if rc < 4:
                                        rhs = xpt3[:, rc * ROWS + kh : rc * ROWS + kh + ROWS, kw : kw + W]
                                    else:
                                        lr = (rc - 4) * ROWS + kh
                                        rhs = xpb3[:, lr : lr + ROWS, kw : kw + W]
                                nc.tensor.matmul(
                                    ps[:], lhsT, rhs,
                                    start=(k == 0), stop=(k == KS * KS - 1),
                                )
                        # psum -> sbuf with per-channel bias add
                        nc.vector.tensor_scalar_add(
                            out_sb[:, half * PIX + rc * NF : half * PIX + (rc + 1) * NF],
                            ps[:],
                            bias_sb[:, half : half + 1],
                        )
                        if last_img and half == 1:
                            # fine-grained tail on the (now idle) sync ring
                            nc.sync.dma_start(
                                y_d[n, 128:256, rc * ROWS : (rc + 1) * ROWS, :]
                                .rearrange("c h w -> c (h w)"),
                                out_sb[:, half * PIX + rc * NF : half * PIX + (rc + 1) * NF],
                            )
                    if not (last_img and half == 1):
                        eng = nc.scalar if half == 0 else nc.gpsimd
                        eng.dma_start(
                            y_d[n, half * 128 : (half + 1) * 128].rearrange("c h w -> c (h w)"),
                            out_sb[:, half * PIX : (half + 1) * PIX],
                        )
    nc.compile()
    return nc


def _get_nc():
    if "nc" not in _CACHE:
        _CACHE["nc"] = _build()
    return _CACHE["nc"]


def _prep_inputs(x, weight, bias):
    # fp16 on host: halves input DMA bytes and drops the on-device casts;
    # same rounding the device cast would apply
    x = np.ascontiguousarray(np.asarray(x, dtype=np.float32).astype(np.float16))
    # [co, ci, kh, kw] -> [ci, half, kh*kw, co_half], half-major so the half-0
    # block is contiguous and can be DMA'd first
    w_t = np.ascontiguousarray(
        np.transpose(np.asarray(weight, dtype=np.float32), (1, 2, 3, 0))
        .reshape(C_IN, KS * KS, 2, 128)
        .transpose(0, 2, 1, 3)
        .astype(np.float16)
    )
    b = np.ascontiguousarray(bias, dtype=np.float32)
    return x, w_t, b


def kernel(x, weight, bias):
    from concourse.bass_utils import run_bass_kernel_spmd

    x, w_t, b = _prep_inputs(x, weight, bias)
    nc = _get_nc()
    in_maps = [
        {"x": x[i * N_PER : (i + 1) * N_PER], "w": w_t, "b": b}
        for i in range(N_CORES)
    ]
    res = run_bass_kernel_spmd(nc, in_maps, list(range(N_CORES)))
    y = np.concatenate([res.results[i]["y"] for i in range(N_CORES)], axis=0)
    return y


# revision 9
# speedup vs baseline: 1.0032x; 1.0032x over previous
"""Conv2d(128->256, 3x3, pad 1, stride 1) on 32x56x56 fp32, for 8 trn2 cores.

Strategy: data-parallel over batch N=32 -> 4 images/core. Per core an
implicit-GEMM conv: C_in=128 is the partition (contraction) dim; for each
(kh, kw) tap a [128ci x 128co] weight tile multiplies a shifted window of the
zero-padded input image held in SBUF, accumulating into PSUM over the 9 taps.
Output rows are processed in chunks of 8 (free dim 8*56=448 <= 512 PSUM bank).
Matmuls run in float16 (~2.6e-4 rel err) with fp32 PSUM accumulate. The 504-
matmul stream runs at the 448-cycle/2.4GHz issue floor, so the win is all in
the head and tail:

Head: weight half-0 is split into its own tiles (taps 0-2, taps 3-8) so the
first LDWEIGHTS gates on a 98KB transfer, not the full weight stream. Image 0
is loaded in 4 pieces split across the sync and gpsimd rings, with chunk 0's
padded rows in a dedicated small tile so its 9 matmuls can start as soon as
x rows 0-8 land (~7.6us) instead of waiting for the whole top half. A short
3-matmul warmup bridges PE busy-ness from ~6.9us for the HAM clock gate.

Tail: the final chunk is drained in four 2-row pieces, two on DVE + two on
ACT (scalar.add), with output DMAs alternating sync/gpsimd rings, so the
last-chunk drain+DMA serialization is ~1.4us instead of ~3us.

Rings: SP(sync) carries x + half-1 fine-grained outputs, ACT(scalar) carries
weights/bias + half-0 outputs, SWDGE(gpsimd) carries image-0 x pieces and
half-1 bulk outputs.
"""
import numpy as np
from contextlib import ExitStack

N_FULL, C_IN, H, W = 32, 128, 56, 56
C_OUT, KS = 256, 3
N_CORES = 8
N_PER = N_FULL // N_CORES          # 4 images per core
HP = H + 2                          # 58 padded
PIX = H * W                         # 3136
ROWS = 8                            # output rows per psum chunk
RC = H // ROWS                      # 7 chunks
NF = ROWS * W                       # 448 free elems per matmul

T_ROWS = 34                         # xpad_top: padded rows 0..33  (chunks 0-3)
B_ROWS = 26                         # xpad_bot: padded rows 32..57 (chunks 4-6)
XT_R = 33                           # x rows 0..32 feed top interior
XT_A = 17                           # images 1-3 first sub-DMA: x rows 0..16
XT_B = XT_R - XT_A                  # images 1-3 second sub-DMA: x rows 17..32
XB_R = 25                           # x rows 31..55 feed bottom interior

# image-0 head pieces
P1_R = 9                            # x rows 0..8   (sync)   -> chunk 0
P2_R = 8                            # x rows 9..16  (gpsimd)
P3_R = 16                           # x rows 17..32 (sync)
Q0_ROWS = 10                        # xq0: padded rows 0..9   (chunk 0)
Q1_ROWS = 26                        # xq1: padded rows 8..33  (chunks 1-3)

_CACHE = {}


def _build():
    import concourse.tile as tile
    from concourse import mybir, bacc

    f32 = mybir.dt.float32
    f16 = mybir.dt.float16

    nc = bacc.Bacc("TRN2", target_bir_lowering=False, debug=False)
    x_d = nc.dram_tensor("x", [N_PER, C_IN, H, W], f16, kind="ExternalInput").ap()
    # host-pretransposed: [ci, half, k, co_half] (half-major, contiguous per half)
    w_d = nc.dram_tensor("w", [C_IN, 2, KS * KS, 128], f16, kind="ExternalInput").ap()
    b_d = nc.dram_tensor("b", [C_OUT], f32, kind="ExternalInput").ap()
    y_d = nc.dram_tensor("y", [N_PER, C_OUT, H, W], f32, kind="ExternalOutput").ap()

    with tile.TileContext(nc) as tc:
        with ExitStack() as ctx:
            wp = ctx.enter_context(tc.tile_pool(name="wp", bufs=1))
            xrawta = ctx.enter_context(tc.tile_pool(name="xrawta", bufs=1))
            xrawtb = ctx.enter_context(tc.tile_pool(name="xrawtb", bufs=1))
            xrawb = ctx.enter_context(tc.tile_pool(name="xrawb", bufs=1))
            xr0b = ctx.enter_context(tc.tile_pool(name="xr0b", bufs=1))
            xq0p = ctx.enter_context(tc.tile_pool(name="xq0p", bufs=1))
            xq1p = ctx.enter_context(tc.tile_pool(name="xq1p", bufs=1))
            xq1bp = ctx.enter_context(tc.tile_pool(name="xq1bp", bufs=1))
            xpadt = ctx.enter_context(tc.tile_pool(name="xpadt", bufs=2))
            xpadb = ctx.enter_context(tc.tile_pool(name="xpadb", bufs=2))
            pp = ctx.enter_context(tc.tile_pool(name="pp", bufs=4, space="PSUM"))
            op = ctx.enter_context(tc.tile_pool(name="op", bufs=2))

            # Weight tiles split per 3 taps; weight pieces ride the sync
            # ring interleaved with image-0's x pieces (the sync ring sustains
            # ~190GB/s from ~8us; the scalar ring is slower in the head).
            w0a = wp.tile([C_IN, 3 * 128], f16)      # half 0, taps 0-2
            w0b1 = wp.tile([C_IN, 3 * 128], f16)     # half 0, taps 3-5
            w0b2 = wp.tile([C_IN, 3 * 128], f16)     # half 0, taps 6-8
            w1 = wp.tile([C_IN, KS * KS * 128], f16)  # half 1, all taps

            # sync ring (fast, low latency): p1 then the three half-0
            # weight pieces -- exactly what the first chunk's warm-rate
            # matmuls need, in need order. Four descriptors max: each
            # dma_start costs ~650ns of queue issue + ~1.8us latency.
            xp1 = xrawta.tile([C_IN, P1_R * W], f16)
            nc.sync.dma_start(xp1[:], x_d[0, :, 0:P1_R, :].rearrange("c h w -> c (h w)"))
            nc.sync.dma_start(
                w0a[:], w_d[:, 0, 0:3].rearrange("ci k co -> ci (k co)")
            )
            nc.sync.dma_start(
                w0b1[:], w_d[:, 0, 3:6].rearrange("ci k co -> ci (k co)")
            )
            nc.sync.dma_start(
                w0b2[:], w_d[:, 0, 6:9].rearrange("ci k co -> ci (k co)")
            )

            # scalar ring: bias, the rest of image-0's top half, weight half 1
            bias_sb = wp.tile([128, 2], f32)
            nc.scalar.dma_start(bias_sb[:], b_d.rearrange("(h p) -> p h", h=2))
            xp2 = xr0b.tile([C_IN, P2_R * W], f16)
            nc.scalar.dma_start(xp2[:], x_d[0, :, P1_R : P1_R + P2_R, :].rearrange("c h w -> c (h w)"))
            xp3 = xrawtb.tile([C_IN, P3_R * W], f16)
            nc.scalar.dma_start(xp3[:], x_d[0, :, 17:33, :].rearrange("c h w -> c (h w)"))
            nc.scalar.dma_start(
                w1[:], w_d[:, 1].rearrange("ci k co -> ci (k co)")
            )

            def lhsT_for(half, k):
                if half == 1:
                    return w1[:, k * 128 : (k + 1) * 128]
                t = (w0a, w0b1, w0b2)[k // 3]
                return t[:, (k % 3) * 128 : (k % 3 + 1) * 128]

            # gpsimd queue: warmup memset first (gates the PE warmup), then
            # the bottom x piece on the SWDGE ring (high latency, needed late),
            # then the image-0 pad memsets (no data deps).
            wu = wp.tile([128, 448], f16)
            nc.gpsimd.memset(wu[:], 0.0)
            xp4 = xrawb.tile([C_IN, XB_R * W], f16)
            nc.gpsimd.dma_start(xp4[:], x_d[0, :, 31 : 31 + XB_R, :].rearrange("c h w -> c (h w)"))

            # PE warmup: 9 dummy matmuls run gaplessly from ~6.9us so the HAM
            # SHORT window sees a full busy window and opens the clock gate at
            # ~10.3us, exactly when the real stream takes over warm.
            wups = pp.tile([128, NF], f32, tag="ps")
            for _ in range(9):
                nc.tensor.matmul(wups[:], wu[:, 0:128], wu[:], start=True, stop=True)

            # image-0 padded tiles, one per chunk-group so each chunk gates
            # only on its own pieces: xq0 = padded rows 0..9 (chunk 0),
            # xq1a = rows 8..17 (chunk 1), xq1b = rows 16..33 (chunks 2-3),
            # xpb0 = rows 32..57 (chunks 4-6).
            xq0 = xq0p.tile([C_IN, Q0_ROWS * HP], f16)
            xq03 = xq0[:].rearrange("p (a b) -> p a b", a=Q0_ROWS)
            nc.gpsimd.memset(xq03[:, 0, :], 0.0)
            nc.gpsimd.memset(xq03[:, 1:Q0_ROWS, 0:1], 0.0)
            nc.gpsimd.memset(xq03[:, 1:Q0_ROWS, HP - 1 : HP], 0.0)
            nc.vector.tensor_copy(
                xq03[:, 1:Q0_ROWS, 1 : 1 + W],
                xp1[:].rearrange("p (a b) -> p a b", a=P1_R),
            )

            xq1a = xq1p.tile([C_IN, 10 * HP], f16)
            xq1a3 = xq1a[:].rearrange("p (a b) -> p a b", a=10)
            nc.gpsimd.memset(xq1a3[:, :, 0:1], 0.0)
            nc.gpsimd.memset(xq1a3[:, :, HP - 1 : HP], 0.0)
            # padded rows 8..17 = x rows 7..16: 2 rows from xp1, 8 from xp2
            nc.vector.tensor_copy(
                xq1a3[:, 0:2, 1 : 1 + W],
                xp1[:].rearrange("p (a b) -> p a b", a=P1_R)[:, 7:9, :],
            )
            nc.vector.tensor_copy(
                xq1a3[:, 2:10, 1 : 1 + W],
                xp2[:].rearrange("p (a b) -> p a b", a=P2_R),
            )

            xq1b = xq1bp.tile([C_IN, 18 * HP], f16)
            xq1b3 = xq1b[:].rearrange("p (a b) -> p a b", a=18)
            nc.gpsimd.memset(xq1b3[:, :, 0:1], 0.0)
            nc.gpsimd.memset(xq1b3[:, :, HP - 1 : HP], 0.0)
            # padded rows 16..33 = x rows 15..32: 2 rows from xp2, 16 from xp3
            nc.vector.tensor_copy(
                xq1b3[:, 0:2, 1 : 1 + W],
                xp2[:].rearrange("p (a b) -> p a b", a=P2_R)[:, 6:8, :],
            )
            nc.vector.tensor_copy(
                xq1b3[:, 2:18, 1 : 1 + W],
                xp3[:].rearrange("p (a b) -> p a b", a=P3_R),
            )

            xpb0 = xpadb.tile([C_IN, B_ROWS * HP], f16)
            xpb03 = xpb0[:].rearrange("p (a b) -> p a b", a=B_ROWS)
            nc.gpsimd.memset(xpb03[:, B_ROWS - 1, :], 0.0)
            nc.gpsimd.memset(xpb03[:, 0 : B_ROWS - 1, 0:1], 0.0)
            nc.gpsimd.memset(xpb03[:, 0 : B_ROWS - 1, HP - 1 : HP], 0.0)
            nc.vector.tensor_copy(
                xpb03[:, 0 : B_ROWS - 1, 1 : 1 + W],
                xp4[:].rearrange("p (a b) -> p a b", a=XB_R),
            )

            for n in range(N_PER):
                if n == 0:
                    xpt3 = None
                    xpb3 = xpb03
                else:
                    # images 1-3: top interior in two slices (pipelined with
                    # the previous image's compute via bufs=1 staging pools)
                    xrta = xrawta.tile([C_IN, XT_A * W], f16)
                    nc.sync.dma_start(xrta[:], x_d[n, :, 0:XT_A, :].rearrange("c h w -> c (h w)"))
                    xrtb = xrawtb.tile([C_IN, XT_B * W], f16)
                    nc.sync.dma_start(xrtb[:], x_d[n, :, XT_A:XT_R, :].rearrange("c h w -> c (h w)"))
                    xrb = xrawb.tile([C_IN, XB_R * W], f16)
                    nc.sync.dma_start(xrb[:], x_d[n, :, 31 : 31 + XB_R, :].rearrange("c h w -> c (h w)"))

                    xpt = xpadt.tile([C_IN, T_ROWS * HP], f16)
                    xpt3 = xpt[:].rearrange("p (a b) -> p a b", a=T_ROWS)
                    nc.vector.memset(xpt3[:, 0, :], 0.0)
                    nc.vector.memset(xpt3[:, 1:T_ROWS, 0:1], 0.0)
                    nc.vector.memset(xpt3[:, 1:T_ROWS, HP - 1 : HP], 0.0)
                    nc.vector.tensor_copy(
                        xpt3[:, 1 : 1 + XT_A, 1 : 1 + W],
                        xrta[:].rearrange("p (a b) -> p a b", a=XT_A),
                    )
                    nc.vector.tensor_copy(
                        xpt3[:, 1 + XT_A : 1 + XT_R, 1 : 1 + W],
                        xrtb[:].rearrange("p (a b) -> p a b", a=XT_B),
                    )

                    xpb = xpadb.tile([C_IN, B_ROWS * HP], f16)
                    xpb3 = xpb[:].rearrange("p (a b) -> p a b", a=B_ROWS)
                    nc.vector.memset(xpb3[:, B_ROWS - 1, :], 0.0)
                    nc.vector.memset(xpb3[:, 0 : B_ROWS - 1, 0:1], 0.0)
                    nc.vector.memset(xpb3[:, 0 : B_ROWS - 1, HP - 1 : HP], 0.0)
                    nc.vector.tensor_copy(
                        xpb3[:, 0 : B_ROWS - 1, 1 : 1 + W],
                        xrb[:].rearrange("p (a b) -> p a b", a=XB_R),
                    )

                out_sb = op.tile([128, 2 * PIX], f32)
                last_img = n == N_PER - 1
                for half in range(2):
                    for rc in range(RC):
                        if last_img and half == 1 and rc == RC - 1:
                            # final chunk as two 4-row PSUM groups: sub-0
                            # drains+stores while sub-1 still computes, so the
                            # post-last-matmul tail covers only 4 rows.
                            HNF = NF // 2      # 224
                            QNF = NF // 4      # 112
                            for sub in range(2):
                                pss = pp.tile([128, HNF], f32)
                                for kh in range(KS):
                                    for kw in range(KS):
                                        k = kh * KS + kw
                                        lr = (rc - 4) * ROWS + sub * 4 + kh
                                        nc.tensor.matmul(
                                            pss[:], lhsT_for(half, k),
                                            xpb3[:, lr : lr + 4, kw : kw + W],
                                            start=(k == 0), stop=(k == KS * KS - 1),
                                        )
                                lo = half * PIX + rc * NF + sub * HNF
                                r0 = rc * ROWS + sub * 4
                                if sub == 0:
                                    nc.vector.tensor_scalar_add(
                                        out_sb[:, lo : lo + HNF], pss[:],
                                        bias_sb[:, half : half + 1],
                                    )
                                    nc.sync.dma_start(
                                        y_d[n, 128:256, r0 : r0 + 4, :]
                                        .rearrange("c h w -> c (h w)"),
                                        out_sb[:, lo : lo + HNF],
                                    )
                                else:
                                    nc.vector.tensor_scalar_add(
                                        out_sb[:, lo : lo + QNF],
                                        pss[:, 0:QNF],
                                        bias_sb[:, half : half + 1],
                                    )
                                    nc.sync.dma_start(
                                        y_d[n, 128:256, r0 : r0 + 2, :]
                                        .rearrange("c h w -> c (h w)"),
                                        out_sb[:, lo : lo + QNF],
                                    )
                                    nc.vector.tensor_scalar_add(
                                        out_sb[:, lo + QNF : lo + HNF],
                                        pss[:, QNF:HNF],
                                        bias_sb[:, half : half + 1],
                                    )
                                    nc.scalar.dma_start(
                                        y_d[n, 128:256, r0 + 2 : r0 + 4, :]
                                        .rearrange("c h w -> c (h w)"),
                                        out_sb[:, lo + QNF : lo + HNF],
                                    )
                            continue
                        ps = pp.tile([128, NF], f32)
                        for kh in range(KS):
                            for kw in range(KS):
                                k = kh * KS + kw
                                lhsT = lhsT_for(half, k)
                                if n == 0:
                                    if rc == 0:
                                        rhs = xq03[:, kh : kh + ROWS, kw : kw + W]
                                    elif rc == 1:
                                        rhs = xq1a3[:, kh : kh + ROWS, kw : kw + W]
                                    elif rc < 4:
                                        lr = (rc - 2) * ROWS + kh
                                        rhs = xq1b3[:, lr : lr + ROWS, kw : kw + W]
                                    else:
                                        lr = (rc - 4) * ROWS + kh
                                        rhs = xpb3[:, lr : lr + ROWS, kw : kw + W]
                                else:
                                    if rc < 4:
                                        rhs = xpt3[:, rc * ROWS + kh : rc * ROWS + kh + ROWS, kw : kw + W]
                                    else:
                                        lr = (rc - 4) * ROWS + kh
                                        rhs = xpb3[:, lr : lr + ROWS, kw : kw + W]
                                nc.tensor.matmul(
                                    ps[:], lhsT, rhs,
                                    start=(k == 0), stop=(k == KS * KS - 1),
                                )
                        # psum -> sbuf with per-channel bias add
                        nc.vector.tensor_scalar_add(
                            out_sb[:, half * PIX + rc * NF : half * PIX + (rc + 1) * NF],
                            ps[:],
                            bias_sb[:, half : half + 1],
                        )
                        if last_img and half == 1:
                            # fine-grained tail on the (now idle) sync ring
                            nc.sync.dma_start(
                                y_d[n, 128:256, rc * ROWS : (rc + 1) * ROWS, :]
                                .rearrange("c h w -> c (h w)"),
                                out_sb[:, half * PIX + rc * NF : half * PIX + (rc + 1) * NF],
                            )
                    if not (last_img and half == 1):
                        eng = nc.scalar if half == 0 else nc.gpsimd
                        eng.dma_start(
                            y_d[n, half * 128 : (half + 1) * 128].rearrange("c h w -> c (h w)"),
                            out_sb[:, half * PIX : (half + 1) * PIX],
                        )
    nc.compile()
    return nc


def _get_nc():
    if "nc" not in _CACHE:
        _CACHE["nc"] = _build()
    return _CACHE["nc"]


def _prep_inputs(x, weight, bias):
    # fp16 on host: halves input DMA bytes and drops the on-device casts;
    # same rounding the device cast would apply
    x = np.ascontiguousarray(np.asarray(x, dtype=np.float32).astype(np.float16))
    # [co, ci, kh, kw] -> [ci, half, kh*kw, co_half], half-major so the half-0
    # block is contiguous and can be DMA'd first
    w_t = np.ascontiguousarray(
        np.transpose(np.asarray(weight, dtype=np.float32), (1, 2, 3, 0))
        .reshape(C_IN, KS * KS, 2, 128)
        .transpose(0, 2, 1, 3)
        .astype(np.float16)
    )
    b = np.ascontiguousarray(bias, dtype=np.float32)
    return x, w_t, b


def kernel(x, weight, bias):
    from concourse.bass_utils import run_bass_kernel_spmd

    x, w_t, b = _prep_inputs(x, weight, bias)
    nc = _get_nc()
    in_maps = [
        {"x": x[i * N_PER : (i + 1) * N_PER], "w": w_t, "b": b}
        for i in range(N_CORES)
    ]
    res = run_bass_kernel_spmd(nc, in_maps, list(range(N_CORES)))
    y = np.concatenate([res.results[i]["y"] for i in range(N_CORES)], axis=0)
    return y


# revision 12
# speedup vs baseline: 1.0168x; 1.0135x over previous
"""Conv2d(128->256, 3x3, pad 1, stride 1) on 32x56x56 fp32, for 8 trn2 cores.

Strategy: data-parallel over batch N=32 -> 4 images/core. Per core an
implicit-GEMM conv: C_in=128 is the partition (contraction) dim; for each
(kh, kw) tap a [128ci x 128co] weight tile multiplies a shifted window of the
zero-padded input image held in SBUF, accumulating into PSUM over the 9 taps.
Output rows are processed in chunks of 8 (free dim 8*56=448 <= 512 PSUM bank).
Matmuls run in float16 (~2.6e-4 rel err) with fp32 PSUM accumulate. The 504-
matmul stream runs at the 448-cycle/2.4GHz issue floor, so the win is all in
the head and tail:

Head: weight half-0 is split into its own tiles (taps 0-2, taps 3-8) so the
first LDWEIGHTS gates on a 98KB transfer, not the full weight stream. Image 0
is loaded in 4 pieces split across the sync and gpsimd rings, with chunk 0's
padded rows in a dedicated small tile so its 9 matmuls can start as soon as
x rows 0-8 land (~7.6us) instead of waiting for the whole top half. A short
3-matmul warmup bridges PE busy-ness from ~6.9us for the HAM clock gate.

Tail: the final chunk is drained in four 2-row pieces, two on DVE + two on
ACT (scalar.add), with output DMAs alternating sync/gpsimd rings, so the
last-chunk drain+DMA serialization is ~1.4us instead of ~3us.

Rings: SP(sync) carries x + half-1 fine-grained outputs, ACT(scalar) carries
weights/bias + half-0 outputs, SWDGE(gpsimd) carries image-0 x pieces and
half-1 bulk outputs.
"""
import numpy as np
from contextlib import ExitStack

N_FULL, C_IN, H, W = 32, 128, 56, 56
C_OUT, KS = 256, 3
N_CORES = 8
N_PER = N_FULL // N_CORES          # 4 images per core
HP = H + 2                          # 58 padded
PIX = H * W                         # 3136
ROWS = 8                            # output rows per psum chunk
RC = H // ROWS                      # 7 chunks
NF = ROWS * W                       # 448 free elems per matmul

T_ROWS = 34                         # xpad_top: padded rows 0..33  (chunks 0-3)
B_ROWS = 26                         # xpad_bot: padded rows 32..57 (chunks 4-6)
XT_R = 33                           # x rows 0..32 feed top interior
XT_A = 17                           # images 1-3 first sub-DMA: x rows 0..16
XT_B = XT_R - XT_A                  # images 1-3 second sub-DMA: x rows 17..32
XB_R = 25                           # x rows 31..55 feed bottom interior

# image-0 head pieces
P1_R = 9                            # x rows 0..8   (sync)   -> chunk 0
P2_R = 8                            # x rows 9..16  (gpsimd)
P3_R = 16                           # x rows 17..32 (sync)
Q0_ROWS = 10                        # xq0: padded rows 0..9   (chunk 0)
Q1_ROWS = 26                        # xq1: padded rows 8..33  (chunks 1-3)

_CACHE = {}


def _build():
    import concourse.tile as tile
    from concourse import mybir, bacc

    f32 = mybir.dt.float32
    f16 = mybir.dt.float16

    nc = bacc.Bacc("TRN2", target_bir_lowering=False, debug=False)
    x_d = nc.dram_tensor("x", [N_PER, C_IN, H, W], f16, kind="ExternalInput").ap()
    # host-pretransposed: [ci, half, k, co_half] (half-major, contiguous per half)
    w_d = nc.dram_tensor("w", [C_IN, 2, KS * KS, 128], f16, kind="ExternalInput").ap()
    b_d = nc.dram_tensor("b", [C_OUT], f32, kind="ExternalInput").ap()
    y_d = nc.dram_tensor("y", [N_PER, C_OUT, H, W], f32, kind="ExternalOutput").ap()

    with tile.TileContext(nc) as tc:
        with ExitStack() as ctx:
            wp = ctx.enter_context(tc.tile_pool(name="wp", bufs=1))
            xrawta = ctx.enter_context(tc.tile_pool(name="xrawta", bufs=1))
            xrawtb = ctx.enter_context(tc.tile_pool(name="xrawtb", bufs=1))
            xrawb = ctx.enter_context(tc.tile_pool(name="xrawb", bufs=1))
            xr0b = ctx.enter_context(tc.tile_pool(name="xr0b", bufs=1))
            xq0p = ctx.enter_context(tc.tile_pool(name="xq0p", bufs=1))
            xq1p = ctx.enter_context(tc.tile_pool(name="xq1p", bufs=1))
            xq1bp = ctx.enter_context(tc.tile_pool(name="xq1bp", bufs=1))
            xpadt = ctx.enter_context(tc.tile_pool(name="xpadt", bufs=2))
            xpadb = ctx.enter_context(tc.tile_pool(name="xpadb", bufs=2))
            pp = ctx.enter_context(tc.tile_pool(name="pp", bufs=4, space="PSUM"))
            op = ctx.enter_context(tc.tile_pool(name="op", bufs=2))

            # Only 3 engine queues can trigger DMAs (sync/SP, scalar/ACT,
            # gpsimd/SWDGE); a queue's 1st descriptor flows at ~8.0us and each
            # later one ~1.8us behind the previous transfer. Allocation:
            #   sync:   w0 (all half-0 taps, lands ~9.6 < first warm LDW),
            #           then w1, then images 1-3 x
            #   scalar: p1 (chunk 0 rows), p2, p4
            #   gpsimd: bias, p3 (+ wu/pad memsets)
            w0 = wp.tile([C_IN, KS * KS * 128], f16)  # half 0, all taps
            w1 = wp.tile([C_IN, KS * KS * 128], f16)  # half 1, all taps
            nc.sync.dma_start(
                w0[:], w_d[:, 0].rearrange("ci k co -> ci (k co)")
            )
            nc.sync.dma_start(
                w1[:], w_d[:, 1].rearrange("ci k co -> ci (k co)")
            )
            xp1 = xrawta.tile([C_IN, P1_R * W], f16)
            nc.scalar.dma_start(xp1[:], x_d[0, :, 0:P1_R, :].rearrange("c h w -> c (h w)"))
            xp2 = xr0b.tile([C_IN, P2_R * W], f16)
            nc.scalar.dma_start(xp2[:], x_d[0, :, P1_R : P1_R + P2_R, :].rearrange("c h w -> c (h w)"))
            wu = wp.tile([128, 448], f16)
            nc.gpsimd.memset(wu[:], 0.0)
            bias_sb = wp.tile([128, 2], f32)
            nc.gpsimd.dma_start(bias_sb[:], b_d.rearrange("(h p) -> p h", h=2))
            xp3 = xrawtb.tile([C_IN, P3_R * W], f16)
            nc.gpsimd.dma_start(xp3[:], x_d[0, :, 17:33, :].rearrange("c h w -> c (h w)"))

            def lhsT_for(half, k):
                t = w1 if half == 1 else w0
                return t[:, k * 128 : (k + 1) * 128]

            # bottom x piece (3rd descriptor on scalar)
            xp4 = xrawb.tile([C_IN, XB_R * W], f16)
            nc.scalar.dma_start(xp4[:], x_d[0, :, 31 : 31 + XB_R, :].rearrange("c h w -> c (h w)"))

            # PE warmup: 9 dummy matmuls run gaplessly from ~6.9us so the HAM
            # SHORT window sees a full busy window and opens the clock gate at
            # ~10.3us, exactly when the real stream takes over warm.
            wups = pp.tile([128, NF], f32, tag="ps")
            for _ in range(9):
                nc.tensor.matmul(wups[:], wu[:, 0:128], wu[:], start=True, stop=True)

            # image-0 padded tiles, one per chunk-group so each chunk gates
            # only on its own pieces: xq0 = padded rows 0..9 (chunk 0),
            # xq1a = rows 8..17 (chunk 1), xq1b = rows 16..33 (chunks 2-3),
            # xpb0 = rows 32..57 (chunks 4-6).
            xq0 = xq0p.tile([C_IN, Q0_ROWS * HP], f16)
            xq03 = xq0[:].rearrange("p (a b) -> p a b", a=Q0_ROWS)
            nc.gpsimd.memset(xq03[:, 0, :], 0.0)
            nc.gpsimd.memset(xq03[:, 1:Q0_ROWS, 0:1], 0.0)
            nc.gpsimd.memset(xq03[:, 1:Q0_ROWS, HP - 1 : HP], 0.0)
            nc.vector.tensor_copy(
                xq03[:, 1:Q0_ROWS, 1 : 1 + W],
                xp1[:].rearrange("p (a b) -> p a b", a=P1_R),
            )

            xq1a = xq1p.tile([C_IN, 10 * HP], f16)
            xq1a3 = xq1a[:].rearrange("p (a b) -> p a b", a=10)
            nc.gpsimd.memset(xq1a3[:, :, 0:1], 0.0)
            nc.gpsimd.memset(xq1a3[:, :, HP - 1 : HP], 0.0)
            # padded rows 8..17 = x rows 7..16: 2 rows from xp1, 8 from xp2
            nc.vector.tensor_copy(
                xq1a3[:, 0:2, 1 : 1 + W],
                xp1[:].rearrange("p (a b) -> p a b", a=P1_R)[:, 7:9, :],
            )
            nc.vector.tensor_copy(
                xq1a3[:, 2:10, 1 : 1 + W],
                xp2[:].rearrange("p (a b) -> p a b", a=P2_R),
            )

            xq1b = xq1bp.tile([C_IN, 18 * HP], f16)
            xq1b3 = xq1b[:].rearrange("p (a b) -> p a b", a=18)
            nc.gpsimd.memset(xq1b3[:, :, 0:1], 0.0)
            nc.gpsimd.memset(xq1b3[:, :, HP - 1 : HP], 0.0)
            # padded rows 16..33 = x rows 15..32: 2 rows from xp2, 16 from xp3
            nc.vector.tensor_copy(
                xq1b3[:, 0:2, 1 : 1 + W],
                xp2[:].rearrange("p (a b) -> p a b", a=P2_R)[:, 6:8, :],
            )
            nc.vector.tensor_copy(
                xq1b3[:, 2:18, 1 : 1 + W],
                xp3[:].rearrange("p (a b) -> p a b", a=P3_R),
            )

            xpb0 = xpadb.tile([C_IN, B_ROWS * HP], f16)
            xpb03 = xpb0[:].rearrange("p (a b) -> p a b", a=B_ROWS)
            nc.gpsimd.memset(xpb03[:, B_ROWS - 1, :], 0.0)
            nc.gpsimd.memset(xpb03[:, 0 : B_ROWS - 1, 0:1], 0.0)
            nc.gpsimd.memset(xpb03[:, 0 : B_ROWS - 1, HP - 1 : HP], 0.0)
            nc.vector.tensor_copy(
                xpb03[:, 0 : B_ROWS - 1, 1 : 1 + W],
                xp4[:].rearrange("p (a b) -> p a b", a=XB_R),
            )

            for n in range(N_PER):
                if n == 0:
                    xpt3 = None
                    xpb3 = xpb03
                else:
                    # images 1-3: top interior in two slices (pipelined with
                    # the previous image's compute via bufs=1 staging pools)
                    xrta = xrawta.tile([C_IN, XT_A * W], f16)
                    nc.sync.dma_start(xrta[:], x_d[n, :, 0:XT_A, :].rearrange("c h w -> c (h w)"))
                    xrtb = xrawtb.tile([C_IN, XT_B * W], f16)
                    nc.sync.dma_start(xrtb[:], x_d[n, :, XT_A:XT_R, :].rearrange("c h w -> c (h w)"))
                    xrb = xrawb.tile([C_IN, XB_R * W], f16)
                    nc.sync.dma_start(xrb[:], x_d[n, :, 31 : 31 + XB_R, :].rearrange("c h w -> c (h w)"))

                    xpt = xpadt.tile([C_IN, T_ROWS * HP], f16)
                    xpt3 = xpt[:].rearrange("p (a b) -> p a b", a=T_ROWS)
                    nc.vector.memset(xpt3[:, 0, :], 0.0)
                    nc.vector.memset(xpt3[:, 1:T_ROWS, 0:1], 0.0)
                    nc.vector.memset(xpt3[:, 1:T_ROWS, HP - 1 : HP], 0.0)
                    nc.vector.tensor_copy(
                        xpt3[:, 1 : 1 + XT_A, 1 : 1 + W],
                        xrta[:].rearrange("p (a b) -> p a b", a=XT_A),
                    )
                    nc.vector.tensor_copy(
                        xpt3[:, 1 + XT_A : 1 + XT_R, 1 : 1 + W],
                        xrtb[:].rearrange("p (a b) -> p a b", a=XT_B),
                    )

                    xpb = xpadb.tile([C_IN, B_ROWS * HP], f16)
                    xpb3 = xpb[:].rearrange("p (a b) -> p a b", a=B_ROWS)
                    nc.vector.memset(xpb3[:, B_ROWS - 1, :], 0.0)
                    nc.vector.memset(xpb3[:, 0 : B_ROWS - 1, 0:1], 0.0)
                    nc.vector.memset(xpb3[:, 0 : B_ROWS - 1, HP - 1 : HP], 0.0)
                    nc.vector.tensor_copy(
                        xpb3[:, 0 : B_ROWS - 1, 1 : 1 + W],
                        xrb[:].rearrange("p (a b) -> p a b", a=XB_R),
                    )

                out_sb = op.tile([128, 2 * PIX], f32)
                last_img = n == N_PER - 1
                for half in range(2):
                    for rc in range(RC):
                        if last_img and half == 1 and rc == RC - 1:
                            # final chunk as two 4-row PSUM groups: sub-0
                            # drains+stores while sub-1 still computes, so the
                            # post-last-matmul tail covers only 4 rows.
                            HNF = NF // 2      # 224
                            QNF = NF // 4      # 112
                            for sub in range(2):
                                pss = pp.tile([128, HNF], f32)
                                for kh in range(KS):
                                    for kw in range(KS):
                                        k = kh * KS + kw
                                        lr = (rc - 4) * ROWS + sub * 4 + kh
                                        nc.tensor.matmul(
                                            pss[:], lhsT_for(half, k),
                                            xpb3[:, lr : lr + 4, kw : kw + W],
                                            start=(k == 0), stop=(k == KS * KS - 1),
                                        )
                                lo = half * PIX + rc * NF + sub * HNF
                                r0 = rc * ROWS + sub * 4
                                if sub == 0:
                                    nc.vector.tensor_scalar_add(
                                        out_sb[:, lo : lo + HNF], pss[:],
                                        bias_sb[:, half : half + 1],
                                    )
                                    nc.sync.dma_start(
                                        y_d[n, 128:256, r0 : r0 + 4, :]
                                        .rearrange("c h w -> c (h w)"),
                                        out_sb[:, lo : lo + HNF],
                                    )
                                else:
                                    nc.vector.tensor_scalar_add(
                                        out_sb[:, lo : lo + QNF],
                                        pss[:, 0:QNF],
                                        bias_sb[:, half : half + 1],
                                    )
                                    nc.sync.dma_start(
                                        y_d[n, 128:256, r0 : r0 + 2, :]
                                        .rearrange("c h w -> c (h w)"),
                                        out_sb[:, lo : lo + QNF],
                                    )
                                    nc.vector.tensor_scalar_add(
                                        out_sb[:, lo + QNF : lo + HNF],
                                        pss[:, QNF:HNF],
                                        bias_sb[:, half : half + 1],
                                    )
                                    nc.scalar.dma_start(
                                        y_d[n, 128:256, r0 + 2 : r0 + 4, :]
                                        .rearrange("c h w -> c (h w)"),
                                        out_sb[:, lo + QNF : lo + HNF],
                                    )
                            continue
                        ps = pp.tile([128, NF], f32)
                        for kh in range(KS):
                            for kw in range(KS):
                                k = kh * KS + kw
                                lhsT = lhsT_for(half, k)
                                if n == 0:
                                    if rc == 0:
                                        rhs = xq03[:, kh : kh + ROWS, kw : kw + W]
                                    elif rc == 1:
                                        rhs = xq1a3[:, kh : kh + ROWS, kw : kw + W]
                                    elif rc < 4:
                                        lr = (rc - 2) * ROWS + kh
                                        rhs = xq1b3[:, lr : lr + ROWS, kw : kw + W]
                                    else:
                                        lr = (rc - 4) * ROWS + kh
                                        rhs = xpb3[:, lr : lr + ROWS, kw : kw + W]
                                else:
                                    if rc < 4:
                                        rhs = xpt3[:, rc * ROWS + kh : rc * ROWS + kh + ROWS, kw : kw + W]
                                    else:
                                        lr = (rc - 4) * ROWS + kh
                                        rhs = xpb3[:, lr : lr + ROWS, kw : kw + W]
                                nc.tensor.matmul(
                                    ps[:], lhsT, rhs,
                                    start=(k == 0), stop=(k == KS * KS - 1),
                                )
                        # psum -> sbuf with per-channel bias add
                        nc.vector.tensor_scalar_add(
                            out_sb[:, half * PIX + rc * NF : half * PIX + (rc + 1) * NF],
                            ps[:],
                            bias_sb[:, half : half + 1],
                        )
                        if last_img and half == 1:
                            # fine-grained tail on the (now idle) sync ring
                            nc.sync.dma_start(
                                y_d[n, 128:256, rc * ROWS : (rc + 1) * ROWS, :]
                                .rearrange("c h w -> c (h w)"),
                                out_sb[:, half * PIX + rc * NF : half * PIX + (rc + 1) * NF],
                            )
                    if not (last_img and half == 1):
                        eng = nc.scalar if half == 0 else nc.gpsimd
                        eng.dma_start(
                            y_d[n, half * 128 : (half + 1) * 128].rearrange("c h w -> c (h w)"),
                            out_sb[:, half * PIX : (half + 1) * PIX],
                        )
    nc.compile()
    return nc


def _get_nc():
    if "nc" not in _CACHE:
        _CACHE["nc"] = _build()
    return _CACHE["nc"]


def _prep_inputs(x, weight, bias):
    # fp16 on host: halves input DMA bytes and drops the on-device casts;
    # same rounding the device cast would apply
    x = np.ascontiguousarray(np.asarray(x, dtype=np.float32).astype(np.float16))
    # [co, ci, kh, kw] -> [ci, half, kh*kw, co_half], half-major so the half-0
    # block is contiguous and can be DMA'd first
    w_t = np.ascontiguousarray(
        np.transpose(np.asarray(weight, dtype=np.float32), (1, 2, 3, 0))
        .reshape(C_IN, KS * KS, 2, 128)
        .transpose(0, 2, 1, 3)
        .astype(np.float16)
    )
    b = np.ascontiguousarray(bias, dtype=np.float32)
    return x, w_t, b


def kernel(x, weight, bias):
    from concourse.bass_utils import run_bass_kernel_spmd

    x, w_t, b = _prep_inputs(x, weight, bias)
    nc = _get_nc()
    in_maps = [
        {"x": x[i * N_PER : (i + 1) * N_PER], "w": w_t, "b": b}
        for i in range(N_CORES)
    ]
    res = run_bass_kernel_spmd(nc, in_maps, list(range(N_CORES)))
    y = np.concatenate([res.results[i]["y"] for i in range(N_CORES)], axis=0)
    return y
